# revision 16
# baseline (speedup 1.0000x reference)
"""MoE top-1 routed expert FFN (8 experts) on 8 Trainium2 NeuronCores.

Strategy: expert parallelism. Core e holds expert e's weights. The host
computes the token->expert permutation (top-1 dispatch is just a gather),
ships each core its tokens transposed (tokens on the matmul free dim),
and the device runs the whole FFN in transposed token space:

    hT = gelu_tanh(w1_tile.T @ xT + b1)        (per 128-wide ff tile)
    yT = sum_ff w2_tile.T @ hT + b2            (accumulated in PSUM)

so w1 ([D, FF]) and w2 ([FF, D]) act as PE stationary operands in their
natural layouts and no on-device transpose is needed. The host scatters
each core's yT back into the full output (tokens are disjoint across
experts, so the source's all-reduce degenerates to a scatter).

Matmul operands are fp16 (fast weight load + full-rate PE, ~5e-4 rel err)
with fp32 PSUM accumulation; the output ships back as fp16 to halve the
writeback DMA. Weights are packed host-side into one [w1 slab | w2 tile]
pack per 128-wide ff tile so each DMA is large and in PE consumption
order, x is packed per token chunk so the first chunk's tokens land
before the second chunk's, and the PE stream is software-pipelined (mm1
of step i+1 issues before mm2 of step i) so the gelu latency on the
scalar engine never stalls the in-order PE queue. The last chunk runs
all of mm1 first (keeping its h tiles in SBUF), then mm2 d-tile-major so
each output tile drains (bias-add + DMA) overlapped with the next
d-tile's accumulation instead of all six serially after the final
matmul. A burst of dummy matmuls at kernel start keeps the PE busy
through the p-state ramp while the first DMAs are in flight, so the real
stream runs at full clock from its first instruction.
"""

import os

import numpy as np

import concourse.mybir as mybir
import concourse.tile as tile
from concourse import bacc, bass_utils

N_CORES = 8
D = 768
FF = 3072
KD = D // 128  # 6
KF = FF // 128  # 24
NPACK = KF  # one ff-tile of (w1 slab | w2 tile) per DMA pack

_compiled = {}


def _maybe_trace():
    """Enable NTFF tracing only when MOE_TRACE=1 and the axon profile hook
    can be installed. The graded path never sets the env var."""
    if not os.environ.get("MOE_TRACE"):
        return False
    try:
        import sys
        import types

        if "antenv.axon_hooks" not in sys.modules:
            mod = types.ModuleType("antenv.axon_hooks")
            _h = [None]
            mod.set_axon_ntff_profile_hook = lambda h: _h.__setitem__(0, h)
            mod.get_axon_ntff_profile_hook = lambda: _h[0]
            sys.modules["antenv.axon_hooks"] = mod
            from trn_agent_boot.trn_boot import _ntff_profile_via_ctypes

            mod.set_axon_ntff_profile_hook(
                _ntff_profile_via_ctypes("/opt/axon/libaxon_pjrt.so")
            )
        return True
    except Exception:
        return False


def _build(chunks):
    """Build + compile the per-core FFN kernel for token chunk sizes `chunks`."""
    f32 = mybir.dt.float32
    f16 = mybir.dt.float16
    gelu = mybir.ActivationFunctionType.Gelu_apprx_tanh
    ident = mybir.ActivationFunctionType.Identity

    nc = bacc.Bacc("TRN2", target_bir_lowering=False, debug=False, num_devices=N_CORES)
    # xp{i}[p, k*Cc + c] = x[token offs[i]+c, k*128 + p]  (per-chunk packing)
    xp_d = [
        nc.dram_tensor(f"xp{i}", [128, KD * Cc], f16, kind="ExternalInput").ap()
        for i, Cc in enumerate(chunks)
    ]
    # wp[ff]: [w1h(ff) | w2(ff)], each half a [128, 768] lhsT slab
    wp_d = nc.dram_tensor("wp", [NPACK, 128, 2 * D], f16, kind="ExternalInput").ap()
    # bp[:, :KF] = b1 tiles, bp[:, KF:KF+KD] = b2 tiles
    bp_d = nc.dram_tensor("bp", [128, KF + KD], f32, kind="ExternalInput").ap()
    yT_d = nc.dram_tensor("yT", [D, sum(chunks)], f16, kind="ExternalOutput").ap()

    nci = len(chunks)
    offs = [sum(chunks[:j]) for j in range(nci)]

    with tile.TileContext(nc) as tc:
        with (
            tc.tile_pool(name="wpool", bufs=1) as wpool,
            tc.tile_pool(name="xpool", bufs=1) as xpool,
            tc.tile_pool(name="hpool", bufs=4) as hpool,
            tc.tile_pool(name="h1pool", bufs=1) as h1pool,
            tc.tile_pool(name="ypool", bufs=6) as ypool,
            tc.tile_pool(name="bpool", bufs=1) as bpool,
            tc.tile_pool(name="phpool", bufs=2, space="PSUM") as phpool,
            tc.tile_pool(name="pypool", bufs=1, space="PSUM") as pypool,
        ):
            # PE warmup: dummy matmuls with no DMA dependency keep the PE busy
            # through the p-state ramp window while input DMAs are in flight,
            # so the real matmul stream starts at full clock. Sized so the
            # burst ends right as the first x/w DMAs complete (~4.5us).
            warm_w = bpool.tile([128, 128], f16, tag="warm")
            nc.vector.memset(warm_w[:], 0.0)
            warm_ps = phpool.tile([128, chunks[0]], f32, tag="ph", name="warm_ps")

            def warm(n, tgt=None):
                tgt = warm_ps[:, :128] if tgt is None else tgt
                for _ in range(n):
                    nc.tensor.matmul(tgt, warm_w[:], warm_w[:], start=True, stop=True)

            warm(40)
            # preload both ACT PWL tables off the critical path
            warm_h = bpool.tile([128, 16], f16, tag="warmh")
            nc.scalar.activation(warm_h[:], warm_w[:, :16], gelu, bias=0.0, scale=1.0)
            nc.scalar.activation(warm_h[:], warm_w[:, :16], ident, bias=0.0, scale=1.0)

            # Input DMAs split across two HWDGE rings so x and the weight
            # stream transfer concurrently: x + bias on the Scalar ring
            # (idle until the output drains), weight packs on the Sync ring
            # in PE consumption order with the first two packs split into
            # w1/w2 halves (mm1 of step 0/1 needs only the w1 half).
            x_sb = [
                xpool.tile([128, KD * Cc], f16, tag=f"x{i}", name=f"x{i}")
                for i, Cc in enumerate(chunks)
            ]
            w_sb = [
                wpool.tile([128, 2 * D], f16, tag=f"wp{i}", name=f"wp{i}")
                for i in range(NPACK)
            ]
            b_sb = bpool.tile([128, KF + KD], f32, tag="b")
            # first-use data split across both rings in consumption order:
            # the two x0 halves stream concurrently, then pack 0's w1 half
            # (sync) and w2 half (scalar) so mm1/mm2 of step 0 unblock as
            # early as the aggregate HBM bandwidth allows
            xh = KD * chunks[0] // 2
            nc.sync.dma_start(x_sb[0][:, :xh], xp_d[0][:, :xh])
            nc.scalar.dma_start(x_sb[0][:, xh:], xp_d[0][:, xh:])
            nc.sync.dma_start(w_sb[0][:, :D], wp_d[0, :, :D])
            nc.scalar.dma_start(w_sb[0][:, D:], wp_d[0, :, D:])
            nc.scalar.dma_start(b_sb[:], bp_d)
            nc.sync.dma_start(w_sb[1][:, :D], wp_d[1, :, :D])
            nc.sync.dma_start(w_sb[1][:, D:], wp_d[1, :, D:])
            # later chunks' x rides the pack stream in quarter-slabs spread
            # after packs 9-12: it is not consumed until the last chunk's
            # mm1 sweep, and by then the PE-need slack behind the pack
            # stream is large enough that the extra bytes never stall it
            xq = []
            for i in range(1, nci):
                n = KD * chunks[i]
                qs = [(n * j // 4, n * (j + 1) // 4) for j in range(4)]
                xq += [(i, a, b) for a, b in qs]
            for i in range(2, NPACK):
                nc.sync.dma_start(w_sb[i][:], wp_d[i, :, :])
                if 9 <= i < 9 + len(xq):
                    ci, a, b = xq[i - 9]
                    nc.sync.dma_start(x_sb[ci][:, a:b], xp_d[ci][:, a:b])

            py = {
                (ci, d): pypool.tile(
                    [128, chunks[ci]], f32, tag=f"py{d}", name=f"py{d}_{ci}"
                )
                for ci in range(nci)
                for d in range(KD)
            }
            h_tiles = {}

            def mm1(ci, ff, persist=False):
                Cc = chunks[ci]
                wt = w_sb[ff]
                ph = phpool.tile([128, Cc], f32, tag="ph", name=f"ph_{ci}_{ff}")
                for k in range(KD):
                    nc.tensor.matmul(
                        ph[:],
                        wt[:, k * 128 : (k + 1) * 128],
                        x_sb[ci][:, k * Cc : (k + 1) * Cc],
                        start=(k == 0),
                        stop=(k == KD - 1),
                    )
                if persist:
                    h_sb = h1pool.tile([128, Cc], f16, tag=f"h1_{ff}", name=f"h1_{ff}")
                else:
                    h_sb = hpool.tile([128, Cc], f16, tag="h", name=f"h_{ci}_{ff}")
                nc.scalar.activation(
                    h_sb[:], ph[:], gelu, bias=b_sb[:, ff : ff + 1], scale=1.0
                )
                h_tiles[(ci, ff)] = h_sb

            def mm2(ci, ff):
                wt = w_sb[ff]
                h_sb = h_tiles.pop((ci, ff))
                for d in range(KD):
                    nc.tensor.matmul(
                        py[(ci, d)][:],
                        wt[:, D + d * 128 : D + (d + 1) * 128],
                        h_sb[:],
                        start=(ff == 0),
                        stop=(ff == KF - 1),
                    )

            def drain_d(ci, d, on_act, split=False):
                """Bias-add py[(ci,d)] into an fp16 SBUF tile and DMA it out.

                split=True halves the tile across DVE+ACT and two DMA rings
                so the chain after the very last matmul is as short as
                possible (it is fully exposed in the kernel tail)."""
                Cc, c0 = chunks[ci], offs[ci]
                b2ap = b_sb[:, KF + d : KF + d + 1]
                row = yT_d[d * 128 : (d + 1) * 128, :]
                y_sb = ypool.tile([128, Cc], f16, tag="y", name=f"y_{ci}_{d}")
                if on_act:
                    nc.scalar.activation(y_sb[:], py[(ci, d)][:], ident, bias=b2ap)
                else:
                    nc.vector.tensor_scalar_add(y_sb[:], py[(ci, d)][:], b2ap)
                ring = nc.sync if (split or d % 2 == 0) else nc.scalar
                ring.dma_start(row[:, c0 : c0 + Cc], y_sb[:])

            # chunks 0..n-2: software-pipelined mm1/mm2 interleave over ff;
            # each chunk's drain happens during the next chunk's first steps
            # (on DVE, since ACT is busy with that chunk's gelus). Filler
            # warm matmuls pad the pipeline-fill points (first gelu latency)
            # so the PE never idles long enough to drop its p-state.
            steps = [(ci, ff) for ci in range(nci - 1) for ff in range(KF)]
            for idx, (ci, ff) in enumerate(steps):
                mm1(ci, ff)
                if idx <= 1:
                    # fill the first-gelu latency with warm matmuls into py
                    # banks that have not started accumulating yet (mm2 of
                    # step 0 overwrites them with start=True afterwards)
                    warm(1, py[(ci, KD - 1)][:, :128])
                    warm(1, py[(ci, KD - 2)][:, :128])
                if idx > 0:
                    mm2(*steps[idx - 1])
            lc = nci - 1

            # last chunk: full mm1 sweep with persistent h tiles (weights are
            # all resident by now, so the PE runs from SBUF). The sweep's
            # first mm1 slots in before the previous chunk's final mm2 so
            # the ph-buffer rotation never exposes the first gelu's latency,
            # and the previous chunk drains behind the first gelus ...
            for ff in range(KF):
                mm1(lc, ff, persist=True)
                if steps and ff == 0:
                    mm2(*steps[-1])
                if steps and ff < KD:
                    drain_d(steps[-1][0], ff, on_act=False)
            # ... then mm2 d-tile-major: each d's 24-step accumulation ends
            # early enough that its bias-add + output DMA overlap the next
            # d's accumulation; only the last d-tile's drain is exposed.
            for d in range(KD):
                wt_h = [h_tiles[(lc, ff)] for ff in range(KF)]
                for ff in range(KF):
                    nc.tensor.matmul(
                        py[(lc, d)][:],
                        w_sb[ff][:, D + d * 128 : D + (d + 1) * 128],
                        wt_h[ff][:],
                        start=(ff == 0),
                        stop=(ff == KF - 1),
                    )
                drain_d(lc, d, on_act=(d % 2 == 1), split=(d == KD - 1))
    nc.compile()
    return nc


def _get_compiled(chunks):
    key = tuple(chunks)
    if key not in _compiled:
        _compiled[key] = _build(list(key))
    return _compiled[key]


def kernel(inputs, dispatch_order, w1, b1, w2, b2):
    x = np.asarray(inputs, dtype=np.float32)
    B, S, Dm = x.shape
    T = B * S
    xf = x.reshape(T, Dm)
    disp = np.asarray(dispatch_order).astype(np.int64)
    w1 = np.asarray(w1, dtype=np.float32)
    b1 = np.asarray(b1, dtype=np.float32)
    w2 = np.asarray(w2, dtype=np.float32)
    b2 = np.asarray(b2, dtype=np.float32)
    E = w1.shape[0]

    counts = np.bincount(disp, minlength=E)
    cmax = max(int(counts.max()), 16)
    # token capacity per core: near-equal chunks of <=512 (PSUM bank limit
    # for fp32 accumulation), multiples of 16, as small as cmax allows
    C = -(-cmax // 16) * 16
    n_chunks = -(-C // 512)
    chunks = []
    rem = C
    for j in range(n_chunks):
        c = -(-(rem // (n_chunks - j)) // 16) * 16
        chunks.append(c)
        rem -= c
    chunks.sort(reverse=True)
    offs = [sum(chunks[:j]) for j in range(len(chunks))]

    order = np.argsort(disp, kind="stable")
    starts = np.concatenate([[0], np.cumsum(counts)])

    in_maps = []
    for e in range(E):
        ids = order[starts[e] : starts[e + 1]]
        xe = np.zeros((C, Dm), dtype=np.float32)
        xe[: len(ids)] = xf[ids]
        m = {}
        for i, (Cc, c0) in enumerate(zip(chunks, offs)):
            xc = xe[c0 : c0 + Cc].reshape(Cc, KD, 128).transpose(2, 1, 0)
            m[f"xp{i}"] = np.ascontiguousarray(
                xc.reshape(128, KD * Cc)
            ).astype(np.float16)
        # w1 in lhsT slab layout: w1h[ff][p, k*128+c] = w1[k*128+p, ff*128+c]
        w1h = (
            w1[e]
            .reshape(KD, 128, KF, 128)
            .transpose(2, 1, 0, 3)
            .reshape(KF, 128, KD * 128)
        )
        w2t = w2[e].reshape(KF, 128, D)
        m["wp"] = np.ascontiguousarray(
            np.concatenate([w1h, w2t], axis=2)
        ).astype(np.float16)
        m["bp"] = np.ascontiguousarray(
            np.concatenate([b1[e].reshape(KF, 128).T, b2[e].reshape(KD, 128).T], axis=1)
        )
        in_maps.append(m)

    nc = _get_compiled(chunks)
    res = None
    for attempt in range(3):
        try:
            res = bass_utils.run_bass_kernel_spmd(
                nc, in_maps, core_ids=list(range(N_CORES)), trace=_maybe_trace()
            )
            break
        except Exception:
            # transient runtime/tunnel hiccups: retry a couple of times
            if attempt == 2:
                raise
            import time

            time.sleep(2.0)
    if res.exec_time_ns is not None:
        print(f"HW exec time: {res.exec_time_ns} ns")
        if res.instructions_and_trace is not None:
            print(f"trace: {res.instructions_and_trace[1]}")

    out = np.zeros((T, Dm), dtype=np.float32)
    for e in range(E):
        ids = order[starts[e] : starts[e + 1]]
        yT = res.results[e]["yT"]
        out[ids] = yT[:, : len(ids)].T.astype(np.float32)
    return out.reshape(B, S, Dm)


# revision 17
# speedup vs baseline: 1.0241x; 1.0241x over previous
"""MoE top-1 routed expert FFN (8 experts) on 8 Trainium2 NeuronCores.

Strategy: expert parallelism. Core e holds expert e's weights. The host
computes the token->expert permutation (top-1 dispatch is just a gather),
ships each core its tokens transposed (tokens on the matmul free dim),
and the device runs the whole FFN in transposed token space:

    hT = gelu_tanh(w1_tile.T @ xT + b1)        (per 128-wide ff tile)
    yT = sum_ff w2_tile.T @ hT + b2            (accumulated in PSUM)

so w1 ([D, FF]) and w2 ([FF, D]) act as PE stationary operands in their
natural layouts and no on-device transpose is needed. The host scatters
each core's yT back into the full output (tokens are disjoint across
experts, so the source's all-reduce degenerates to a scatter).

Matmul operands are fp16 (fast weight load + full-rate PE, ~5e-4 rel err)
with fp32 PSUM accumulation; the output ships back as fp16 to halve the
writeback DMA. Weights are packed host-side into one [w1 slab | w2 tile]
pack per 128-wide ff tile so each DMA is large and in PE consumption
order, x is packed per token chunk so the first chunk's tokens land
before the second chunk's, and the PE stream is software-pipelined (mm1
of step i+1 issues before mm2 of step i) so the gelu latency on the
scalar engine never stalls the in-order PE queue. The last chunk runs
all of mm1 first (keeping its h tiles in SBUF), then mm2 d-tile-major so
each output tile drains (bias-add + DMA) overlapped with the next
d-tile's accumulation instead of all six serially after the final
matmul. A burst of dummy matmuls at kernel start keeps the PE busy
through the p-state ramp while the first DMAs are in flight, so the real
stream runs at full clock from its first instruction.
"""

import os

import numpy as np

import concourse.mybir as mybir
import concourse.tile as tile
from concourse import bacc, bass_utils

N_CORES = 8
D = 768
FF = 3072
KD = D // 128  # 6
KF = FF // 128  # 24
NPACK = KF  # one ff-tile of (w1 slab | w2 tile) per DMA pack

_compiled = {}


def _maybe_trace():
    """Enable NTFF tracing only when MOE_TRACE=1 and the axon profile hook
    can be installed. The graded path never sets the env var."""
    if not os.environ.get("MOE_TRACE"):
        return False
    try:
        import sys
        import types

        if "antenv.axon_hooks" not in sys.modules:
            mod = types.ModuleType("antenv.axon_hooks")
            _h = [None]
            mod.set_axon_ntff_profile_hook = lambda h: _h.__setitem__(0, h)
            mod.get_axon_ntff_profile_hook = lambda: _h[0]
            sys.modules["antenv.axon_hooks"] = mod
            from trn_agent_boot.trn_boot import _ntff_profile_via_ctypes

            mod.set_axon_ntff_profile_hook(
                _ntff_profile_via_ctypes("/opt/axon/libaxon_pjrt.so")
            )
        return True
    except Exception:
        return False


def _build(chunks):
    """Build + compile the per-core FFN kernel for token chunk sizes `chunks`."""
    f32 = mybir.dt.float32
    f16 = mybir.dt.float16
    gelu = mybir.ActivationFunctionType.Gelu_apprx_tanh
    ident = mybir.ActivationFunctionType.Identity

    nc = bacc.Bacc("TRN2", target_bir_lowering=False, debug=False, num_devices=N_CORES)
    # xp{i}[p, k*Cc + c] = x[token offs[i]+c, k*128 + p]  (per-chunk packing)
    xp_d = [
        nc.dram_tensor(f"xp{i}", [128, KD * Cc], f16, kind="ExternalInput").ap()
        for i, Cc in enumerate(chunks)
    ]
    # wp[ff]: [w1h(ff) | w2(ff)], each half a [128, 768] lhsT slab
    wp_d = nc.dram_tensor("wp", [NPACK, 128, 2 * D], f16, kind="ExternalInput").ap()
    # bp[:, :KF] = b1 tiles, bp[:, KF:KF+KD] = b2 tiles
    bp_d = nc.dram_tensor("bp", [128, KF + KD], f32, kind="ExternalInput").ap()
    yT_d = nc.dram_tensor("yT", [D, sum(chunks)], f16, kind="ExternalOutput").ap()

    nci = len(chunks)
    offs = [sum(chunks[:j]) for j in range(nci)]

    with tile.TileContext(nc) as tc:
        with (
            tc.tile_pool(name="wpool", bufs=1) as wpool,
            tc.tile_pool(name="xpool", bufs=1) as xpool,
            tc.tile_pool(name="hpool", bufs=4) as hpool,
            tc.tile_pool(name="h1pool", bufs=1) as h1pool,
            tc.tile_pool(name="ypool", bufs=6) as ypool,
            tc.tile_pool(name="bpool", bufs=1) as bpool,
            tc.tile_pool(name="phpool", bufs=2, space="PSUM") as phpool,
            tc.tile_pool(name="pypool", bufs=1, space="PSUM") as pypool,
        ):
            # PE warmup: dummy matmuls with no DMA dependency keep the PE busy
            # through the p-state ramp window while input DMAs are in flight,
            # so the real matmul stream starts at full clock. Sized so the
            # burst ends right as the first x/w DMAs complete (~4.5us).
            warm_w = bpool.tile([128, 128], f16, tag="warm")
            nc.vector.memset(warm_w[:], 0.0)
            warm_ps = phpool.tile([128, chunks[0]], f32, tag="ph", name="warm_ps")

            def warm(n, tgt=None):
                tgt = warm_ps[:, :128] if tgt is None else tgt
                for _ in range(n):
                    nc.tensor.matmul(tgt, warm_w[:], warm_w[:], start=True, stop=True)

            warm(45)
            # preload both ACT PWL tables off the critical path
            warm_h = bpool.tile([128, 16], f16, tag="warmh")
            nc.scalar.activation(warm_h[:], warm_w[:, :16], gelu, bias=0.0, scale=1.0)
            nc.scalar.activation(warm_h[:], warm_w[:, :16], ident, bias=0.0, scale=1.0)

            # Input DMAs split across two HWDGE rings so x and the weight
            # stream transfer concurrently: x + bias on the Scalar ring
            # (idle until the output drains), weight packs on the Sync ring
            # in PE consumption order with the first two packs split into
            # w1/w2 halves (mm1 of step 0/1 needs only the w1 half).
            x_sb = [
                xpool.tile([128, KD * Cc], f16, tag=f"x{i}", name=f"x{i}")
                for i, Cc in enumerate(chunks)
            ]
            w_sb = [
                wpool.tile([128, 2 * D], f16, tag=f"wp{i}", name=f"wp{i}")
                for i in range(NPACK)
            ]
            b_sb = bpool.tile([128, KF + KD], f32, tag="b")
            nc.scalar.dma_start(x_sb[0][:], xp_d[0])
            nc.scalar.dma_start(b_sb[:], bp_d)
            for i in range(2):
                nc.sync.dma_start(w_sb[i][:, :D], wp_d[i, :, :D])
                nc.sync.dma_start(w_sb[i][:, D:], wp_d[i, :, D:])
            # later chunks' x rides the pack stream in quarter-slabs spread
            # after packs 9-12: it is not consumed until the last chunk's
            # mm1 sweep, and by then the PE-need slack behind the pack
            # stream is large enough that the extra bytes never stall it
            xq = []
            for i in range(1, nci):
                n = KD * chunks[i]
                qs = [(n * j // 4, n * (j + 1) // 4) for j in range(4)]
                xq += [(i, a, b) for a, b in qs]
            for i in range(2, NPACK):
                nc.sync.dma_start(w_sb[i][:], wp_d[i, :, :])
                if 9 <= i < 9 + len(xq):
                    ci, a, b = xq[i - 9]
                    nc.sync.dma_start(x_sb[ci][:, a:b], xp_d[ci][:, a:b])

            py = {
                (ci, d): pypool.tile(
                    [128, chunks[ci]], f32, tag=f"py{d}", name=f"py{d}_{ci}"
                )
                for ci in range(nci)
                for d in range(KD)
            }
            h_tiles = {}

            def mm1(ci, ff, persist=False):
                Cc = chunks[ci]
                wt = w_sb[ff]
                ph = phpool.tile([128, Cc], f32, tag="ph", name=f"ph_{ci}_{ff}")
                for k in range(KD):
                    nc.tensor.matmul(
                        ph[:],
                        wt[:, k * 128 : (k + 1) * 128],
                        x_sb[ci][:, k * Cc : (k + 1) * Cc],
                        start=(k == 0),
                        stop=(k == KD - 1),
                    )
                if persist:
                    h_sb = h1pool.tile([128, Cc], f16, tag=f"h1_{ff}", name=f"h1_{ff}")
                else:
                    h_sb = hpool.tile([128, Cc], f16, tag="h", name=f"h_{ci}_{ff}")
                nc.scalar.activation(
                    h_sb[:], ph[:], gelu, bias=b_sb[:, ff : ff + 1], scale=1.0
                )
                h_tiles[(ci, ff)] = h_sb

            def mm2(ci, ff):
                wt = w_sb[ff]
                h_sb = h_tiles.pop((ci, ff))
                for d in range(KD):
                    nc.tensor.matmul(
                        py[(ci, d)][:],
                        wt[:, D + d * 128 : D + (d + 1) * 128],
                        h_sb[:],
                        start=(ff == 0),
                        stop=(ff == KF - 1),
                    )

            def drain_d(ci, d, on_act, split=False):
                """Bias-add py[(ci,d)] into an fp16 SBUF tile and DMA it out.

                split=True halves the tile across DVE+ACT and two DMA rings
                so the chain after the very last matmul is as short as
                possible (it is fully exposed in the kernel tail)."""
                Cc, c0 = chunks[ci], offs[ci]
                b2ap = b_sb[:, KF + d : KF + d + 1]
                row = yT_d[d * 128 : (d + 1) * 128, :]
                y_sb = ypool.tile([128, Cc], f16, tag="y", name=f"y_{ci}_{d}")
                if on_act:
                    nc.scalar.activation(y_sb[:], py[(ci, d)][:], ident, bias=b2ap)
                else:
                    nc.vector.tensor_scalar_add(y_sb[:], py[(ci, d)][:], b2ap)
                ring = nc.sync if (split or d % 2 == 0) else nc.scalar
                ring.dma_start(row[:, c0 : c0 + Cc], y_sb[:])

            # chunks 0..n-2: software-pipelined mm1/mm2 interleave over ff;
            # each chunk's drain happens during the next chunk's first steps
            # (on DVE, since ACT is busy with that chunk's gelus). Filler
            # warm matmuls pad the pipeline-fill points (first gelu latency)
            # so the PE never idles long enough to drop its p-state.
            steps = [(ci, ff) for ci in range(nci - 1) for ff in range(KF)]
            for idx, (ci, ff) in enumerate(steps):
                mm1(ci, ff)
                if idx <= 1:
                    # fill the first-gelu latency with warm matmuls into py
                    # banks that have not started accumulating yet (mm2 of
                    # step 0 overwrites them with start=True afterwards)
                    warm(1, py[(ci, KD - 1)][:, :128])
                    warm(1, py[(ci, KD - 2)][:, :128])
                if idx > 0:
                    mm2(*steps[idx - 1])
            lc = nci - 1

            # last chunk: full mm1 sweep with persistent h tiles (weights are
            # all resident by now, so the PE runs from SBUF). The sweep's
            # first mm1 slots in before the previous chunk's final mm2 so
            # the ph-buffer rotation never exposes the first gelu's latency,
            # and the previous chunk drains behind the first gelus ...
            for ff in range(KF):
                mm1(lc, ff, persist=True)
                if steps and ff == 0:
                    mm2(*steps[-1])
                if steps and ff < KD:
                    drain_d(steps[-1][0], ff, on_act=False)
            # ... then mm2 d-tile-major: each d's 24-step accumulation ends
            # early enough that its bias-add + output DMA overlap the next
            # d's accumulation; only the last d-tile's drain is exposed.
            for d in range(KD):
                wt_h = [h_tiles[(lc, ff)] for ff in range(KF)]
                for ff in range(KF):
                    nc.tensor.matmul(
                        py[(lc, d)][:],
                        w_sb[ff][:, D + d * 128 : D + (d + 1) * 128],
                        wt_h[ff][:],
                        start=(ff == 0),
                        stop=(ff == KF - 1),
                    )
                drain_d(lc, d, on_act=(d % 2 == 1), split=(d == KD - 1))
    nc.compile()
    return nc


def _get_compiled(chunks):
    key = tuple(chunks)
    if key not in _compiled:
        _compiled[key] = _build(list(key))
    return _compiled[key]


def kernel(inputs, dispatch_order, w1, b1, w2, b2):
    x = np.asarray(inputs, dtype=np.float32)
    B, S, Dm = x.shape
    T = B * S
    xf = x.reshape(T, Dm)
    disp = np.asarray(dispatch_order).astype(np.int64)
    w1 = np.asarray(w1, dtype=np.float32)
    b1 = np.asarray(b1, dtype=np.float32)
    w2 = np.asarray(w2, dtype=np.float32)
    b2 = np.asarray(b2, dtype=np.float32)
    E = w1.shape[0]

    counts = np.bincount(disp, minlength=E)
    cmax = max(int(counts.max()), 16)
    # token capacity per core: near-equal chunks of <=512 (PSUM bank limit
    # for fp32 accumulation), multiples of 16, as small as cmax allows
    C = -(-cmax // 16) * 16
    n_chunks = -(-C // 512)
    chunks = []
    rem = C
    for j in range(n_chunks):
        c = -(-(rem // (n_chunks - j)) // 16) * 16
        chunks.append(c)
        rem -= c
    chunks.sort(reverse=True)
    offs = [sum(chunks[:j]) for j in range(len(chunks))]

    order = np.argsort(disp, kind="stable")
    starts = np.concatenate([[0], np.cumsum(counts)])

    in_maps = []
    for e in range(E):
        ids = order[starts[e] : starts[e + 1]]
        xe = np.zeros((C, Dm), dtype=np.float32)
        xe[: len(ids)] = xf[ids]
        m = {}
        for i, (Cc, c0) in enumerate(zip(chunks, offs)):
            xc = xe[c0 : c0 + Cc].reshape(Cc, KD, 128).transpose(2, 1, 0)
            m[f"xp{i}"] = np.ascontiguousarray(
                xc.reshape(128, KD * Cc)
            ).astype(np.float16)
        # w1 in lhsT slab layout: w1h[ff][p, k*128+c] = w1[k*128+p, ff*128+c]
        w1h = (
            w1[e]
            .reshape(KD, 128, KF, 128)
            .transpose(2, 1, 0, 3)
            .reshape(KF, 128, KD * 128)
        )
        w2t = w2[e].reshape(KF, 128, D)
        m["wp"] = np.ascontiguousarray(
            np.concatenate([w1h, w2t], axis=2)
        ).astype(np.float16)
        m["bp"] = np.ascontiguousarray(
            np.concatenate([b1[e].reshape(KF, 128).T, b2[e].reshape(KD, 128).T], axis=1)
        )
        in_maps.append(m)

    nc = _get_compiled(chunks)
    res = None
    for attempt in range(3):
        try:
            res = bass_utils.run_bass_kernel_spmd(
                nc, in_maps, core_ids=list(range(N_CORES)), trace=_maybe_trace()
            )
            break
        except Exception:
            # transient runtime/tunnel hiccups: retry a couple of times
            if attempt == 2:
                raise
            import time

            time.sleep(2.0)
    if res.exec_time_ns is not None:
        print(f"HW exec time: {res.exec_time_ns} ns")
        if res.instructions_and_trace is not None:
            print(f"trace: {res.instructions_and_trace[1]}")

    out = np.zeros((T, Dm), dtype=np.float32)
    for e in range(E):
        ids = order[starts[e] : starts[e + 1]]
        yT = res.results[e]["yT"]
        out[ids] = yT[:, : len(ids)].T.astype(np.float32)
    return out.reshape(B, S, Dm)
